# revision 10
# baseline (speedup 1.0000x reference)
"""ADiGCNConv distributed Trainium2 kernel (8 NeuronCores).

Strategy
--------
Node-sharded: core k owns destination nodes [k*N/8, (k+1)*N/8).
The directed-GCN normalization is separable:
    out_nei = diag(o_inv) . A  . diag(i_inv) . x
    in_nei  = diag(i_inv) . A^T. diag(o_inv) . x
so the host prescales x into two bf16 gather tables (y_i = i_inv*x,
y_o = o_inv*x, each split into two <=32768-row halves for int16 gather
indices) and the device does, per destination tile of 64 nodes:
  dma_gather of the source rows for each edge (dest-sorted, chunked
  into 128-edge chunks) -> one-hot selection matrices built on DVE via
  broadcast is_equal against an iota -> TensorE matmul segment-sum
  accumulated in PSUM (feature-major) -> fused dense epilogue
  (degree-gated filter, softmax gates, three weight matmuls, bias via
  rank-3 matmul, PE transpose) -> DMA out.
Everything after aggregation is node-local; tables are replicated per
core so no collectives are needed.
"""

import sys

sys.path.insert(0, "/opt/trn_rl_repo")

import numpy as np
import ml_dtypes

import os

import concourse.bacc as bacc
import concourse.mybir as mybir
from concourse.tile import TileContext
from concourse.bass_utils import run_bass_kernel_spmd

P = 128
D = 128
TILE = 64          # destination nodes per PSUM tile
ST = 6             # tiles per stage
CHUNK = 128        # edges per matmul chunk
NCORES = 8

BF16 = mybir.dt.bfloat16
F32 = mybir.dt.float32
I16 = mybir.dt.int16


def _ceil(a, b):
    return (a + b - 1) // b


class Plan:
    """Shared (SPMD-uniform) program structure, data-independent across cores
    (built from the max chunk counts over all cores)."""

    def __init__(self, n, npc):
        self.N = n
        self.NPC = npc
        self.HALF = _ceil(n, 2)
        assert self.HALF <= 32768, "int16 gather index limit"
        self.NT = _ceil(npc, TILE)
        self.NTP = self.NT * TILE
        self.NS = _ceil(self.NT, ST)
        # filled later:
        self.nch = None          # [2, NT, 2] chunks per (dir, tile, half)
        self.gbase = None        # [2, NT, 2] global chunk index of group start
        self.totch = 0
        self.calls = []          # per stage: list of dicts
        self.gcols = 0

    def finalize(self, counts):
        """counts: [ncores, 2, NT, 2] edge counts."""
        mx = counts.max(axis=0)                      # [2, NT, 2]
        nch = _ceil(mx, CHUNK)
        # ensure every (dir, tile) has >= 1 chunk so PSUM gets initialized
        empty = nch.sum(axis=2) == 0                 # [2, NT]
        nch[:, :, 0][empty] = 1
        self.nch = nch
        # global chunk ordering: stage -> dir -> half -> tile -> chunk
        gbase = np.zeros((2, self.NT, 2), np.int64)
        g = 0
        coloff = 0
        self.calls = []
        for s in range(self.NS):
            t0, t1 = s * ST, min((s + 1) * ST, self.NT)
            stage_calls = []
            seg0 = g
            for d in range(2):
                dirinfo = {"halves": [], "ch": 0, "mslots": {}}
                mslot = 0
                for h in range(2):
                    ch = int(nch[d, t0:t1, h].sum())
                    dirinfo["halves"].append(
                        {"ch": ch, "coloff": coloff, "g0": g}
                    )
                    for t in range(t0, t1):
                        gbase[d, t, h] = g
                        dirinfo["mslots"][(t, h)] = mslot
                        g += int(nch[d, t, h])
                        mslot += int(nch[d, t, h])
                    coloff += ch * 8
                dirinfo["ch"] = mslot
                stage_calls.append(dirinfo)
            self.calls.append(
                {"t0": t0, "t1": t1, "dirs": stage_calls, "seg0": seg0, "segn": g - seg0}
            )
        self.gbase = gbase
        self.totch = g
        self.gcols = coloff
        self.max_ch_dir = max(c["dirs"][d]["ch"] for c in self.calls for d in range(2))
        self.max_seg = max(c["segn"] for c in self.calls)


def preprocess(x, edge_index, in_degree, out_degree,
               out_deg_mask, out_deg_mask_bias, in_deg_mask, in_deg_mask_bias,
               W_src, b_src, W_dst, b_dst, W_out_f, b_out_f, W_in_f, b_in_f,
               W_fc, b_fc, in_deg_table, out_deg_table, ncores=NCORES):
    n = x.shape[0]
    npc = n // ncores
    assert npc * ncores == n
    plan = Plan(n, npc)

    row = np.asarray(edge_index[0], np.int64)
    col = np.asarray(edge_index[1], np.int64)
    e = row.shape[0]

    deg_out = np.bincount(row, minlength=n).astype(np.float32)
    deg_in = np.bincount(col, minlength=n).astype(np.float32)
    o_inv = np.where(deg_out > 0, 1.0 / np.sqrt(np.maximum(deg_out, 1e-12)), 0.0).astype(np.float32)
    i_inv = np.where(deg_in > 0, 1.0 / np.sqrt(np.maximum(deg_in, 1e-12)), 0.0).astype(np.float32)

    xf = np.asarray(x, np.float32)
    y_i = (xf * i_inv[:, None]).astype(ml_dtypes.bfloat16)   # gathered for out_nei
    y_o = (xf * o_inv[:, None]).astype(ml_dtypes.bfloat16)   # gathered for in_nei
    H = plan.HALF
    tables = {
        "yi0": np.ascontiguousarray(y_i[:H]),
        "yi1": np.ascontiguousarray(y_i[H:]),
        "yo0": np.ascontiguousarray(y_o[:H]),
        "yo1": np.ascontiguousarray(y_o[H:]),
    }

    # per-core, per-dir edge lists
    percore = []
    counts = np.zeros((ncores, 2, plan.NT, 2), np.int64)
    for k in range(ncores):
        entry = []
        for d in range(2):
            dst = row if d == 0 else col
            src = col if d == 0 else row
            m = (dst >= k * npc) & (dst < (k + 1) * npc)
            dl = dst[m] - k * npc
            sr = src[m]
            tile = dl // TILE
            half = (sr >= H).astype(np.int64)
            seg = dl % TILE
            gid = tile * 2 + half
            cnt = np.bincount(gid, minlength=plan.NT * 2).reshape(plan.NT, 2)
            counts[k, d] = cnt
            entry.append((dl, sr, tile, half, seg, gid))
        percore.append(entry)
    plan.finalize(counts)

    # degree-gated filter per-node terms (host lookup of tiny tables)
    w_out_vec = (out_deg_table.astype(np.float64) @ W_out_f[0].astype(np.float64))
    w_in_vec = (in_deg_table.astype(np.float64) @ W_in_f[0].astype(np.float64))
    t_out_all = (w_out_vec[np.asarray(out_degree)] + float(b_out_f[0])).astype(np.float32)
    t_in_all = (w_in_vec[np.asarray(in_degree)] + float(b_in_f[0])).astype(np.float32)

    # weights (device constants, shared across cores)
    wf2 = np.stack([W_out_f[0], W_in_f[0]], axis=1).astype(ml_dtypes.bfloat16)  # [128,2]
    wmain = np.concatenate(
        [W_src.T, W_dst.T, 0.5 * W_fc.T], axis=1
    ).astype(ml_dtypes.bfloat16)                                                # [128,384]
    b3rows = np.stack([b_src, b_dst, 0.5 * b_fc], axis=0).astype(np.float32)    # [3,128]
    b3 = b3rows.astype(ml_dtypes.bfloat16)
    iota = np.tile(np.arange(TILE, dtype=np.float32), (P, 1)).astype(ml_dtypes.bfloat16)
    ident = np.eye(P, dtype=np.float32)
    ones = np.ones((1, P), dtype=ml_dtypes.bfloat16)
    consts = {"wf2": wf2, "wmain": wmain, "b3": b3, "iota": iota,
              "ident": ident, "ones": ones}

    # per-core input arrays
    in_maps = []
    for k in range(ncores):
        seg_flat = np.full(plan.totch * CHUNK, -1.0, np.float32)
        idx_flat = np.zeros(plan.totch * CHUNK, np.int64)
        for d in range(2):
            dl, sr, tile, half, seg, gid = percore[k][d]
            order = np.argsort(gid, kind="stable")
            gids = gid[order]
            # rank within group
            cnt = counts[k, d].reshape(-1)
            starts = np.zeros(plan.NT * 2, np.int64)
            starts[1:] = np.cumsum(cnt)[:-1]
            rank = np.arange(gids.shape[0]) - starts[gids]
            gb = plan.gbase[d].reshape(-1)  # [NT*2]
            pos = gb[gids] * CHUNK + rank
            seg_flat[pos] = seg[order]
            idx_flat[pos] = sr[order] - half[order] * H
        gseg = np.ascontiguousarray(
            seg_flat.reshape(plan.totch, CHUNK).T
        ).astype(ml_dtypes.bfloat16)
        # gather index layout per call: [16, ch*8] wrapped, replicated x8
        gidx = np.zeros((P, plan.gcols), np.int16)
        for s in range(plan.NS):
            for d in range(2):
                for h in range(2):
                    hinfo = plan.calls[s]["dirs"][d]["halves"][h]
                    ch, co, g0 = hinfo["ch"], hinfo["coloff"], hinfo["g0"]
                    if ch == 0:
                        continue
                    vals = idx_flat[g0 * CHUNK: (g0 + ch) * CHUNK]
                    arr = vals.reshape(ch * 8, 16).T.astype(np.int16)
                    for g in range(8):
                        gidx[g * 16:(g + 1) * 16, co: co + ch * 8] = arr

        nsl = slice(k * npc, (k + 1) * npc)
        nodedat = np.zeros((plan.NTP, 8), np.float32)
        nodedat[:npc, 0] = o_inv[nsl]
        nodedat[:npc, 1] = i_inv[nsl]
        nodedat[:npc, 2] = t_out_all[nsl]
        nodedat[:npc, 3] = t_in_all[nsl]
        nodedat[:npc, 4] = np.asarray(out_deg_mask, np.float32)[nsl]
        nodedat[:npc, 5] = np.asarray(out_deg_mask_bias, np.float32)[nsl]
        nodedat[:npc, 6] = np.asarray(in_deg_mask, np.float32)[nsl]
        nodedat[:npc, 7] = np.asarray(in_deg_mask_bias, np.float32)[nsl]

        xT = np.zeros((P, plan.NTP), np.float32)
        xT[:, :npc] = xf[nsl].T
        xT = xT.astype(ml_dtypes.bfloat16)

        im = {"gseg": gseg, "gidx": gidx, "nodedat": nodedat, "xT": xT}
        im.update(tables)
        im.update(consts)
        in_maps.append(im)
    return plan, in_maps


PH = int(os.environ.get("GNN_PHASE", "9"))


class StopBuild(Exception):
    pass


def build_kernel(plan):
    nc = bacc.Bacc("TRN2", target_bir_lowering=False, debug=False)
    H, H2 = plan.HALF, plan.N - plan.HALF
    tabs = {}
    for nm, rows in (("yi0", H), ("yi1", H2), ("yo0", H), ("yo1", H2)):
        tabs[nm] = nc.dram_tensor(nm, [rows, D], BF16, kind="ExternalInput")
    gseg_d = nc.dram_tensor("gseg", [P, plan.totch], BF16, kind="ExternalInput")
    gidx_d = nc.dram_tensor("gidx", [P, plan.gcols], I16, kind="ExternalInput")
    ndat_d = nc.dram_tensor("nodedat", [plan.NTP, 8], F32, kind="ExternalInput")
    xT_d = nc.dram_tensor("xT", [P, plan.NTP], BF16, kind="ExternalInput")
    wf2_d = nc.dram_tensor("wf2", [D, 2], BF16, kind="ExternalInput")
    wmain_d = nc.dram_tensor("wmain", [D, 3 * D], BF16, kind="ExternalInput")
    b3_d = nc.dram_tensor("b3", [3, D], BF16, kind="ExternalInput")
    iota_d = nc.dram_tensor("iota", [P, TILE], BF16, kind="ExternalInput")
    ident_d = nc.dram_tensor("ident", [P, P], F32, kind="ExternalInput")
    ones_d = nc.dram_tensor("ones", [1, P], BF16, kind="ExternalInput")
    out_d = nc.dram_tensor("out", [plan.NPC, D], F32, kind="ExternalOutput")
    cout_d = nc.dram_tensor("cout", [plan.NPC, 1], F32, kind="ExternalOutput")
    cin_d = nc.dram_tensor("cin", [plan.NPC, 1], F32, kind="ExternalOutput")

    max_cols = max(
        sum(h["ch"] for d in c["dirs"] for h in d["halves"]) * 8 for c in plan.calls
    )

    with TileContext(nc) as tc:
        with (
            tc.tile_pool(name="const", bufs=1) as cpool,
            tc.tile_pool(name="stream", bufs=2) as spool,
            tc.tile_pool(name="nei", bufs=4 * ST) as neipool,
            tc.tile_pool(name="small", bufs=2) as smpool,
            tc.tile_pool(name="ops", bufs=3) as opool,
            tc.tile_pool(name="ps_mm", bufs=4, space="PSUM") as ps_mm,
            tc.tile_pool(name="ps_misc", bufs=4, space="PSUM") as ps_misc,
        ):
            # resident constants
            wf2 = cpool.tile([D, 2], BF16, tag="wf2")
            wmain = cpool.tile([D, 3 * D], BF16, tag="wmain")
            b3 = cpool.tile([3, D], BF16, tag="b3")
            iota = cpool.tile([P, TILE], BF16, tag="iota")
            ident = cpool.tile([P, P], F32, tag="ident")
            ones = cpool.tile([1, P], BF16, tag="ones")
            xT = cpool.tile([P, plan.NTP], BF16, tag="xT")
            ndat = cpool.tile([TILE, plan.NT * 8], F32, tag="ndat")
            call_o = cpool.tile([TILE, plan.NT], F32, tag="call_o")
            call_i = cpool.tile([TILE, plan.NT], F32, tag="call_i")
            for sb, dr in ((wf2, wf2_d), (wmain, wmain_d), (b3, b3_d),
                           (iota, iota_d), (ident, ident_d), (ones, ones_d),
                           (xT, xT_d)):
                nc.sync.dma_start(sb[:], dr[:])
            nc.sync.dma_start(
                ndat[:].rearrange("p (t v) -> p t v", v=8),
                ndat_d[:].rearrange("(t p) v -> p t v", p=TILE),
            )

            ndat_v = ndat[:].rearrange("p (t v) -> p t v", v=8)

            for s in range(plan.NS):
                c = plan.calls[s]
                t0, t1 = c["t0"], c["t1"]
                T_ = t1 - t0
                # stage streaming inputs
                col0 = c["dirs"][0]["halves"][0]["coloff"]
                ncols = sum(h["ch"] for dd in c["dirs"] for h in dd["halves"]) * 8
                idx_sb = spool.tile([P, max_cols], I16, tag="idx")
                nc.sync.dma_start(idx_sb[:, :ncols], gidx_d[:, col0: col0 + ncols])
                seg_sb = spool.tile([P, plan.max_seg], BF16, tag="seg")
                nc.sync.dma_start(
                    seg_sb[:, : c["segn"]],
                    gseg_d[:, c["seg0"]: c["seg0"] + c["segn"]],
                )

                m_sb = []
                s_sb = []
                for dd in range(2):
                    dinfo = c["dirs"][dd]
                    ch_dir = dinfo["ch"]
                    mt = spool.tile([P, plan.max_ch_dir * CHUNK], BF16, tag=f"m{dd}")
                    m_sb.append(mt)
                    for h in range(2):
                        hi = dinfo["halves"][h]
                        if hi["ch"] == 0:
                            continue
                        tab = tabs[("yi0", "yi1", "yo0", "yo1")[dd * 2 + h]]
                        moff = (hi["g0"] - dinfo["halves"][0]["g0"]) if h else 0
                        nidx = hi["ch"] * CHUNK
                        if PH < 1:
                            continue
                        nc.gpsimd.dma_gather(
                            mt[:, moff * CHUNK: (moff + hi["ch"]) * CHUNK]
                            .rearrange("p (c e) -> p c e", e=D),
                            tab[:],
                            idx_sb[:, hi["coloff"] - col0: hi["coloff"] - col0 + hi["ch"] * 8],
                            nidx,
                            nidx,
                            D,
                            single_packet=False,
                        )
                    # S build for the whole (stage, dir)
                    st = spool.tile([P, plan.max_ch_dir * TILE], BF16, tag=f"s{dd}")
                    s_sb.append(st)
                    soff = dinfo["halves"][0]["g0"] - c["seg0"]
                    if PH < 2:
                        nc.vector.memset(st[:, : ch_dir * TILE], 0.0)
                        continue
                    nc.vector.tensor_tensor(
                        out=st[:, : ch_dir * TILE].rearrange("p (c d) -> p c d", d=TILE),
                        in0=seg_sb[:, soff: soff + ch_dir]
                        .unsqueeze(2).to_broadcast([P, ch_dir, TILE]),
                        in1=iota[:].unsqueeze(1).to_broadcast([P, ch_dir, TILE]),
                        op=mybir.AluOpType.is_equal,
                    )

                # phase A: spmm + filter per tile
                stg = smpool.tile([TILE, 4 * ST], F32, tag="stg")
                neis = {}
                if PH < 3:
                    continue
                for t in range(t0, t1):
                    tl = t - t0
                    for dd in range(2):
                        dinfo = c["dirs"][dd]
                        pst = ps_mm.tile([P, TILE], F32, tag="ps_mm")
                        mms = []
                        for h in range(2):
                            nchv = int(plan.nch[dd, t, h])
                            if nchv == 0:
                                continue
                            base = dinfo["mslots"][(t, h)]
                            mms.extend(range(base, base + nchv))
                        for i, m in enumerate(mms):
                            nc.tensor.matmul(
                                out=pst[:],
                                lhsT=m_sb[dd][:, m * CHUNK: (m + 1) * CHUNK],
                                rhs=s_sb[dd][:, m * TILE: (m + 1) * TILE],
                                start=(i == 0),
                                stop=(i == len(mms) - 1),
                            )
                        nb = neipool.tile([P, TILE], BF16, tag="nei")
                        nc.scalar.activation(
                            out=nb[:], in_=pst[:],
                            func=mybir.ActivationFunctionType.Copy,
                        )
                        neis[(tl, dd)] = nb
                    if PH < 4:
                        continue
                    psf = ps_misc.tile([TILE, 4], F32, tag="ps_misc")
                    nc.tensor.matmul(out=psf[:, 0:1], lhsT=neis[(tl, 0)][:],
                                     rhs=wf2[:, 0:1], start=True, stop=True)
                    nc.tensor.matmul(out=psf[:, 1:2], lhsT=neis[(tl, 1)][:],
                                     rhs=wf2[:, 1:2], start=True, stop=True)
                    nc.tensor.matmul(out=psf[:, 2:4],
                                     lhsT=xT[:, t * TILE: (t + 1) * TILE],
                                     rhs=wf2[:], start=True, stop=True)
                    nc.vector.tensor_copy(out=stg[:, 4 * tl: 4 * tl + 4], in_=psf[:])

                # phase B: gates (batched over tiles in stage)
                if PH < 5:
                    continue
                stg_v = stg[:].rearrange("p (t v) -> p t v", v=4)[:, :T_, :]
                gst = smpool.tile([TILE, 5 * ST], F32, tag="gst")
                gst_v = gst[:].rearrange("p (t v) -> p t v", v=5)[:, :T_, :]
                nd = ndat_v[:, t0:t1, :]
                scr = smpool.tile([TILE, 4 * ST], F32, tag="scr")
                scr_v = scr[:].rearrange("p (t v) -> p t v", v=4)[:, :T_, :]
                MUL, ADD, SUB = (mybir.AluOpType.mult, mybir.AluOpType.add,
                                 mybir.AluOpType.subtract)

                def tt(out, a, b, op, eng=None):
                    (eng or nc.vector).tensor_tensor(out=out, in0=a, in1=b, op=op)

                # c_dir = inv*raw - xw + t
                for dd in range(2):
                    tt(scr_v[:, :, dd], stg_v[:, :, dd], nd[:, :, dd], MUL)
                    tt(scr_v[:, :, dd], scr_v[:, :, dd], stg_v[:, :, 2 + dd], SUB)
                    tt(scr_v[:, :, dd], scr_v[:, :, dd], nd[:, :, 2 + dd], ADD)
                    nc.scalar.activation(
                        out=scr_v[:, :, 2 + dd], in_=scr_v[:, :, dd],
                        func=mybir.ActivationFunctionType.Exp,
                    )
                tt(gst_v[:, :, 0], scr_v[:, :, 2], scr_v[:, :, 3], ADD)
                nc.vector.reciprocal(out=gst_v[:, :, 1], in_=gst_v[:, :, 0])
                for dd in range(2):
                    # C = exp/sum ; Cm = C*mask + mask_bias
                    tt(scr_v[:, :, dd], scr_v[:, :, 2 + dd], gst_v[:, :, 1], MUL)
                    tt(scr_v[:, :, dd], scr_v[:, :, dd], nd[:, :, 4 + 2 * dd], MUL)
                    tt(gst_v[:, :, 2 + dd], scr_v[:, :, dd], nd[:, :, 5 + 2 * dd], ADD)
                    # gain = Cm * inv
                    tt(gst_v[:, :, dd], gst_v[:, :, 2 + dd], nd[:, :, dd], MUL)
                nc.vector.memset(gst_v[:, :, 4], 1.0)
                nc.vector.tensor_copy(out=call_o[:, t0:t1], in_=gst_v[:, :, 2])
                nc.vector.tensor_copy(out=call_i[:, t0:t1], in_=gst_v[:, :, 3])

                if PH < 6:
                    continue
                # phase C: per-tile output
                for t in range(t0, t1):
                    tl = t - t0
                    # transpose this tile's 5 gate columns; transpose
                    # outputs must land at PSUM partition 0, so use three
                    # transposes into disjoint column ranges of one tile
                    gt_ps = ps_misc.tile([3, 3 * TILE], F32, tag="ps_misc")
                    nc.tensor.transpose(out=gt_ps[0:1, 0:TILE],
                                        in_=gst[:, 5 * tl: 5 * tl + 1],
                                        identity=ident[:TILE, :TILE])
                    nc.tensor.transpose(out=gt_ps[0:1, TILE: 2 * TILE],
                                        in_=gst[:, 5 * tl + 1: 5 * tl + 2],
                                        identity=ident[:TILE, :TILE])
                    nc.tensor.transpose(out=gt_ps[0:3, 2 * TILE: 3 * TILE],
                                        in_=gst[:, 5 * tl + 2: 5 * tl + 5],
                                        identity=ident[:TILE, :TILE])
                    gt = opool.tile([3, 3 * TILE], BF16, tag="gt")
                    nc.scalar.activation(out=gt[:], in_=gt_ps[:],
                                         func=mybir.ActivationFunctionType.Copy)
                    bc = ps_misc.tile([P, P], F32, tag="ps_misc")
                    for dd in range(2):
                        nc.tensor.matmul(
                            out=bc[:, dd * TILE: (dd + 1) * TILE],
                            lhsT=ones[0:1, :],
                            rhs=gt[0:1, dd * TILE: (dd + 1) * TILE],
                            start=True, stop=True,
                        )
                    ng = []
                    for dd in range(2):
                        g = opool.tile([P, TILE], BF16, tag=f"ng{dd}")
                        tt(g[:], neis[(tl, dd)][:],
                           bc[:, dd * TILE: (dd + 1) * TILE], MUL)
                        ng.append(g)
                    pso = ps_misc.tile([P, TILE], F32, tag="ps_misc")
                    nc.tensor.matmul(out=pso[:], lhsT=wmain[:, 0:D], rhs=ng[0][:],
                                     start=True, stop=False)
                    nc.tensor.matmul(out=pso[:], lhsT=wmain[:, D:2 * D], rhs=ng[1][:],
                                     start=False, stop=False)
                    nc.tensor.matmul(out=pso[:], lhsT=wmain[:, 2 * D:3 * D],
                                     rhs=xT[:, t * TILE: (t + 1) * TILE],
                                     start=False, stop=False)
                    nc.tensor.matmul(out=pso[:], lhsT=b3[:],
                                     rhs=gt[0:3, 2 * TILE: 3 * TILE],
                                     start=False, stop=True)
                    if PH < 7:
                        continue
                    osb = opool.tile([P, TILE], F32, tag="osb")
                    nc.scalar.activation(out=osb[:], in_=pso[:],
                                         func=mybir.ActivationFunctionType.Copy)
                    tr = ps_misc.tile([TILE, P], F32, tag="ps_misc")
                    nc.tensor.transpose(out=tr[:], in_=osb[:], identity=ident[:])
                    onode = opool.tile([TILE, P], F32, tag="onode")
                    nc.vector.tensor_copy(out=onode[:], in_=tr[:])
                    rows = min(TILE, plan.NPC - t * TILE)
                    nc.sync.dma_start(out_d[t * TILE: t * TILE + rows, :],
                                      onode[:rows, :])

            # C_in / C_out outputs
            nfull = plan.NPC // TILE
            for cal, dr in (((call_o, cout_d), (call_i, cin_d)) if PH >= 8 else ()):
                nc.sync.dma_start(
                    dr[: nfull * TILE, 0].rearrange("(t p) -> p t", p=TILE),
                    cal[:, :nfull],
                )
                rem = plan.NPC - nfull * TILE
                if rem:
                    nc.sync.dma_start(
                        dr[nfull * TILE:, 0].rearrange("(t p) -> p t", p=rem),
                        cal[:rem, nfull: nfull + 1],
                    )
    nc.compile()
    return nc


def run(inputs, trace=False, tmpdir=None):
    plan, in_maps = preprocess(**inputs)
    nc = build_kernel(plan)
    res = run_bass_kernel_spmd(nc, in_maps, core_ids=list(range(NCORES)),
                               trace=trace, tmpdir=tmpdir)
    return plan, res


def kernel(**inputs):
    plan, res = run(inputs)
    n, npc = plan.N, plan.NPC
    out = np.empty((n, D), np.float32)
    c_in = np.empty((n, 1), np.float32)
    c_out = np.empty((n, 1), np.float32)
    for k in range(NCORES):
        r = res.results[k]
        out[k * npc: (k + 1) * npc] = r["out"]
        c_out[k * npc: (k + 1) * npc] = r["cout"]
        c_in[k * npc: (k + 1) * npc] = r["cin"]
    return out, c_in, c_out


def assemble(plan, res):
    n, npc = plan.N, plan.NPC
    out = np.empty((n, D), np.float32)
    c_in = np.empty((n, 1), np.float32)
    c_out = np.empty((n, 1), np.float32)
    for k in range(NCORES):
        r = res.results[k]
        out[k * npc: (k + 1) * npc] = r["out"]
        c_out[k * npc: (k + 1) * npc] = r["cout"]
        c_in[k * npc: (k + 1) * npc] = r["cin"]
    return out, c_in, c_out


# revision 11
# speedup vs baseline: 2.8268x; 2.8268x over previous
"""ADiGCNConv distributed Trainium2 kernel (8 NeuronCores).

Strategy
--------
Node-sharded: core k owns destination nodes [k*N/8, (k+1)*N/8).
The directed-GCN normalization is separable:
    out_nei = diag(o_inv) . A  . diag(i_inv) . x
    in_nei  = diag(i_inv) . A^T. diag(o_inv) . x
so the host prescales x into two bf16 gather tables (y_i = i_inv*x,
y_o = o_inv*x, each split into two <=32768-row halves for int16 gather
indices) and the device does, per destination tile of 64 nodes:
  dma_gather of the source rows for each edge (dest-sorted, chunked
  into 128-edge chunks) -> one-hot selection matrices built on DVE via
  broadcast is_equal against an iota -> TensorE matmul segment-sum
  accumulated in PSUM (feature-major) -> fused dense epilogue
  (degree-gated filter, softmax gates, three weight matmuls, bias via
  rank-3 matmul, PE transpose) -> DMA out.
Everything after aggregation is node-local; tables are replicated per
core so no collectives are needed.
"""

import sys

sys.path.insert(0, "/opt/trn_rl_repo")

import numpy as np
import ml_dtypes

import os

import concourse.bacc as bacc
import concourse.mybir as mybir
from concourse.tile import TileContext
from concourse.bass_utils import run_bass_kernel_spmd

P = 128
D = 128
TILE = 64          # destination nodes per PSUM tile
ST = 6             # tiles per stage
CHUNK = 128        # edges per matmul chunk
NCORES = 8

BF16 = mybir.dt.bfloat16
F32 = mybir.dt.float32
I16 = mybir.dt.int16


def _ceil(a, b):
    return (a + b - 1) // b


class Plan:
    """Shared (SPMD-uniform) program structure, data-independent across cores
    (built from the max chunk counts over all cores)."""

    def __init__(self, n, npc):
        self.N = n
        self.NPC = npc
        self.HALF = _ceil(n, 2)
        assert self.HALF <= 32768, "int16 gather index limit"
        self.NT = _ceil(npc, TILE)
        self.NTP = self.NT * TILE
        self.NS = _ceil(self.NT, ST)
        # filled later:
        self.nch = None          # [2, NT, 2] chunks per (dir, tile, half)
        self.gbase = None        # [2, NT, 2] global chunk index of group start
        self.totch = 0
        self.calls = []          # per stage: list of dicts
        self.gcols = 0

    def finalize(self, counts):
        """counts: [ncores, 2, NT, 2] edge counts."""
        mx = counts.max(axis=0)                      # [2, NT, 2]
        nch = _ceil(mx, CHUNK)
        # ensure every (dir, tile) has >= 1 chunk so PSUM gets initialized
        empty = nch.sum(axis=2) == 0                 # [2, NT]
        nch[:, :, 0][empty] = 1
        self.nch = nch
        # global chunk ordering: stage -> dir -> half -> tile -> chunk
        gbase = np.zeros((2, self.NT, 2), np.int64)
        g = 0
        coloff = 0
        self.calls = []
        for s in range(self.NS):
            t0, t1 = s * ST, min((s + 1) * ST, self.NT)
            stage_calls = []
            seg0 = g
            for d in range(2):
                dirinfo = {"halves": [], "ch": 0, "mslots": {}}
                mslot = 0
                for h in range(2):
                    ch = int(nch[d, t0:t1, h].sum())
                    dirinfo["halves"].append(
                        {"ch": ch, "coloff": coloff, "g0": g}
                    )
                    for t in range(t0, t1):
                        gbase[d, t, h] = g
                        dirinfo["mslots"][(t, h)] = mslot
                        g += int(nch[d, t, h])
                        mslot += int(nch[d, t, h])
                    coloff += ch * 8
                dirinfo["ch"] = mslot
                stage_calls.append(dirinfo)
            self.calls.append(
                {"t0": t0, "t1": t1, "dirs": stage_calls, "seg0": seg0, "segn": g - seg0}
            )
        self.gbase = gbase
        self.totch = g
        self.gcols = coloff
        self.max_ch_dir = max(c["dirs"][d]["ch"] for c in self.calls for d in range(2))
        self.max_seg = max(c["segn"] for c in self.calls)


def preprocess(x, edge_index, in_degree, out_degree,
               out_deg_mask, out_deg_mask_bias, in_deg_mask, in_deg_mask_bias,
               W_src, b_src, W_dst, b_dst, W_out_f, b_out_f, W_in_f, b_in_f,
               W_fc, b_fc, in_deg_table, out_deg_table, ncores=NCORES):
    n = x.shape[0]
    npc = n // ncores
    assert npc * ncores == n
    plan = Plan(n, npc)

    row = np.asarray(edge_index[0], np.int64)
    col = np.asarray(edge_index[1], np.int64)
    e = row.shape[0]

    deg_out = np.bincount(row, minlength=n).astype(np.float32)
    deg_in = np.bincount(col, minlength=n).astype(np.float32)
    o_inv = np.where(deg_out > 0, 1.0 / np.sqrt(np.maximum(deg_out, 1e-12)), 0.0).astype(np.float32)
    i_inv = np.where(deg_in > 0, 1.0 / np.sqrt(np.maximum(deg_in, 1e-12)), 0.0).astype(np.float32)

    xf = np.asarray(x, np.float32)
    y_i = (xf * i_inv[:, None]).astype(ml_dtypes.bfloat16)   # gathered for out_nei
    y_o = (xf * o_inv[:, None]).astype(ml_dtypes.bfloat16)   # gathered for in_nei
    H = plan.HALF
    tables = {
        "yi0": np.ascontiguousarray(y_i[:H]),
        "yi1": np.ascontiguousarray(y_i[H:]),
        "yo0": np.ascontiguousarray(y_o[:H]),
        "yo1": np.ascontiguousarray(y_o[H:]),
    }

    # per-core, per-dir edge lists
    percore = []
    counts = np.zeros((ncores, 2, plan.NT, 2), np.int64)
    for k in range(ncores):
        entry = []
        for d in range(2):
            dst = row if d == 0 else col
            src = col if d == 0 else row
            m = (dst >= k * npc) & (dst < (k + 1) * npc)
            dl = dst[m] - k * npc
            sr = src[m]
            tile = dl // TILE
            half = (sr >= H).astype(np.int64)
            seg = dl % TILE
            gid = tile * 2 + half
            cnt = np.bincount(gid, minlength=plan.NT * 2).reshape(plan.NT, 2)
            counts[k, d] = cnt
            entry.append((dl, sr, tile, half, seg, gid))
        percore.append(entry)
    plan.finalize(counts)

    # degree-gated filter per-node terms (host lookup of tiny tables)
    w_out_vec = (out_deg_table.astype(np.float64) @ W_out_f[0].astype(np.float64))
    w_in_vec = (in_deg_table.astype(np.float64) @ W_in_f[0].astype(np.float64))
    t_out_all = (w_out_vec[np.asarray(out_degree)] + float(b_out_f[0])).astype(np.float32)
    t_in_all = (w_in_vec[np.asarray(in_degree)] + float(b_in_f[0])).astype(np.float32)

    # weights (device constants, shared across cores)
    wf2 = np.stack([W_out_f[0], W_in_f[0]], axis=1).astype(ml_dtypes.bfloat16)  # [128,2]
    wmain = np.concatenate(
        [W_src.T, W_dst.T, 0.5 * W_fc.T], axis=1
    ).astype(ml_dtypes.bfloat16)                                                # [128,384]
    b3rows = np.stack([b_src, b_dst, 0.5 * b_fc], axis=0).astype(np.float32)    # [3,128]
    b3 = b3rows.astype(ml_dtypes.bfloat16)
    iota = np.tile(np.arange(TILE, dtype=np.float32), (P, 1)).astype(ml_dtypes.bfloat16)
    ident = np.eye(P, dtype=np.float32)
    ones = np.ones((1, P), dtype=ml_dtypes.bfloat16)
    consts = {"wf2": wf2, "wmain": wmain, "b3": b3, "iota": iota,
              "ident": ident, "ones": ones}

    # per-core input arrays
    in_maps = []
    for k in range(ncores):
        seg_flat = np.full(plan.totch * CHUNK, -1.0, np.float32)
        idx_flat = np.zeros(plan.totch * CHUNK, np.int64)
        for d in range(2):
            dl, sr, tile, half, seg, gid = percore[k][d]
            order = np.argsort(gid, kind="stable")
            gids = gid[order]
            # rank within group
            cnt = counts[k, d].reshape(-1)
            starts = np.zeros(plan.NT * 2, np.int64)
            starts[1:] = np.cumsum(cnt)[:-1]
            rank = np.arange(gids.shape[0]) - starts[gids]
            gb = plan.gbase[d].reshape(-1)  # [NT*2]
            pos = gb[gids] * CHUNK + rank
            seg_flat[pos] = seg[order]
            idx_flat[pos] = sr[order] - half[order] * H
        gseg = np.ascontiguousarray(
            seg_flat.reshape(plan.totch, CHUNK).T
        ).astype(ml_dtypes.bfloat16)
        # gather index layout per call: [16, ch*8] wrapped, replicated x8
        gidx = np.zeros((P, plan.gcols), np.int16)
        for s in range(plan.NS):
            for d in range(2):
                for h in range(2):
                    hinfo = plan.calls[s]["dirs"][d]["halves"][h]
                    ch, co, g0 = hinfo["ch"], hinfo["coloff"], hinfo["g0"]
                    if ch == 0:
                        continue
                    vals = idx_flat[g0 * CHUNK: (g0 + ch) * CHUNK]
                    arr = vals.reshape(ch * 8, 16).T.astype(np.int16)
                    for g in range(8):
                        gidx[g * 16:(g + 1) * 16, co: co + ch * 8] = arr

        nsl = slice(k * npc, (k + 1) * npc)
        nodedat = np.zeros((plan.NTP, 8), np.float32)
        nodedat[:npc, 0] = o_inv[nsl]
        nodedat[:npc, 1] = i_inv[nsl]
        nodedat[:npc, 2] = t_out_all[nsl]
        nodedat[:npc, 3] = t_in_all[nsl]
        nodedat[:npc, 4] = np.asarray(out_deg_mask, np.float32)[nsl]
        nodedat[:npc, 5] = np.asarray(out_deg_mask_bias, np.float32)[nsl]
        nodedat[:npc, 6] = np.asarray(in_deg_mask, np.float32)[nsl]
        nodedat[:npc, 7] = np.asarray(in_deg_mask_bias, np.float32)[nsl]

        xT = np.zeros((P, plan.NTP), np.float32)
        xT[:, :npc] = xf[nsl].T
        xT = xT.astype(ml_dtypes.bfloat16)

        im = {"gseg": gseg, "gidx": gidx, "nodedat": nodedat, "xT": xT}
        im.update(tables)
        im.update(consts)
        in_maps.append(im)
    return plan, in_maps


PH = int(os.environ.get("GNN_PHASE", "9"))


class StopBuild(Exception):
    pass


def build_kernel(plan):
    nc = bacc.Bacc("TRN2", target_bir_lowering=False, debug=False,
                   num_swdge_queues=4)
    H, H2 = plan.HALF, plan.N - plan.HALF
    tabs = {}
    for nm, rows in (("yi0", H), ("yi1", H2), ("yo0", H), ("yo1", H2)):
        tabs[nm] = nc.dram_tensor(nm, [rows, D], BF16, kind="ExternalInput")
    gseg_d = nc.dram_tensor("gseg", [P, plan.totch], BF16, kind="ExternalInput")
    gidx_d = nc.dram_tensor("gidx", [P, plan.gcols], I16, kind="ExternalInput")
    ndat_d = nc.dram_tensor("nodedat", [plan.NTP, 8], F32, kind="ExternalInput")
    xT_d = nc.dram_tensor("xT", [P, plan.NTP], BF16, kind="ExternalInput")
    wf2_d = nc.dram_tensor("wf2", [D, 2], BF16, kind="ExternalInput")
    wmain_d = nc.dram_tensor("wmain", [D, 3 * D], BF16, kind="ExternalInput")
    b3_d = nc.dram_tensor("b3", [3, D], BF16, kind="ExternalInput")
    iota_d = nc.dram_tensor("iota", [P, TILE], BF16, kind="ExternalInput")
    ident_d = nc.dram_tensor("ident", [P, P], F32, kind="ExternalInput")
    ones_d = nc.dram_tensor("ones", [1, P], BF16, kind="ExternalInput")
    out_d = nc.dram_tensor("out", [plan.NPC, D], F32, kind="ExternalOutput")
    cout_d = nc.dram_tensor("cout", [plan.NPC, 1], F32, kind="ExternalOutput")
    cin_d = nc.dram_tensor("cin", [plan.NPC, 1], F32, kind="ExternalOutput")

    max_cols = max(
        sum(h["ch"] for d in c["dirs"] for h in d["halves"]) * 8 for c in plan.calls
    )

    with TileContext(nc) as tc:
        with (
            tc.tile_pool(name="const", bufs=1) as cpool,
            tc.tile_pool(name="stream", bufs=2) as spool,
            tc.tile_pool(name="nei", bufs=4 * ST) as neipool,
            tc.tile_pool(name="small", bufs=2) as smpool,
            tc.tile_pool(name="ops", bufs=3) as opool,
            tc.tile_pool(name="ps_mm", bufs=4, space="PSUM") as ps_mm,
            tc.tile_pool(name="ps_misc", bufs=4, space="PSUM") as ps_misc,
        ):
            # resident constants
            wf2 = cpool.tile([D, 2], BF16, tag="wf2")
            wmain = cpool.tile([D, 3 * D], BF16, tag="wmain")
            b3 = cpool.tile([3, D], BF16, tag="b3")
            iota = cpool.tile([P, TILE], BF16, tag="iota")
            ident = cpool.tile([P, P], F32, tag="ident")
            ones = cpool.tile([1, P], BF16, tag="ones")
            xT = cpool.tile([P, plan.NTP], BF16, tag="xT")
            ndat = cpool.tile([TILE, plan.NT * 8], F32, tag="ndat")
            call_o = cpool.tile([TILE, plan.NT], F32, tag="call_o")
            call_i = cpool.tile([TILE, plan.NT], F32, tag="call_i")
            for sb, dr in ((wf2, wf2_d), (wmain, wmain_d), (b3, b3_d),
                           (iota, iota_d), (ident, ident_d), (ones, ones_d),
                           (xT, xT_d)):
                nc.sync.dma_start(sb[:], dr[:])
            nc.sync.dma_start(
                ndat[:].rearrange("p (t v) -> p t v", v=8),
                ndat_d[:].rearrange("(t p) v -> p t v", p=TILE),
            )

            ndat_v = ndat[:].rearrange("p (t v) -> p t v", v=8)

            for s in range(plan.NS):
                c = plan.calls[s]
                t0, t1 = c["t0"], c["t1"]
                T_ = t1 - t0
                # stage streaming inputs
                col0 = c["dirs"][0]["halves"][0]["coloff"]
                ncols = sum(h["ch"] for dd in c["dirs"] for h in dd["halves"]) * 8
                idx_sb = spool.tile([P, max_cols], I16, tag="idx")
                nc.sync.dma_start(idx_sb[:, :ncols], gidx_d[:, col0: col0 + ncols])
                seg_sb = spool.tile([P, plan.max_seg], BF16, tag="seg")
                nc.sync.dma_start(
                    seg_sb[:, : c["segn"]],
                    gseg_d[:, c["seg0"]: c["seg0"] + c["segn"]],
                )

                m_sb = []
                s_sb = []
                qn = 0
                for dd in range(2):
                    dinfo = c["dirs"][dd]
                    ch_dir = dinfo["ch"]
                    mt = spool.tile([P, plan.max_ch_dir * CHUNK], BF16, tag=f"m{dd}")
                    m_sb.append(mt)
                    for h in range(2):
                        hi = dinfo["halves"][h]
                        if hi["ch"] == 0:
                            continue
                        tab = tabs[("yi0", "yi1", "yo0", "yo1")[dd * 2 + h]]
                        moff = (hi["g0"] - dinfo["halves"][0]["g0"]) if h else 0
                        nidx = hi["ch"] * CHUNK
                        if PH < 1:
                            continue
                        nc.gpsimd.dma_gather(
                            mt[:, moff * CHUNK: (moff + hi["ch"]) * CHUNK]
                            .rearrange("p (c e) -> p c e", e=D),
                            tab[:],
                            idx_sb[:, hi["coloff"] - col0: hi["coloff"] - col0 + hi["ch"] * 8],
                            nidx,
                            nidx,
                            D,
                            single_packet=False,
                            queue_num=qn % 4,
                        )
                        qn += 1
                    # S build for the whole (stage, dir)
                    st = spool.tile([P, plan.max_ch_dir * TILE], BF16, tag=f"s{dd}")
                    s_sb.append(st)
                    soff = dinfo["halves"][0]["g0"] - c["seg0"]
                    if PH < 2:
                        nc.vector.memset(st[:, : ch_dir * TILE], 0.0)
                        continue
                    nc.vector.tensor_tensor(
                        out=st[:, : ch_dir * TILE].rearrange("p (c d) -> p c d", d=TILE),
                        in0=seg_sb[:, soff: soff + ch_dir]
                        .unsqueeze(2).to_broadcast([P, ch_dir, TILE]),
                        in1=iota[:].unsqueeze(1).to_broadcast([P, ch_dir, TILE]),
                        op=mybir.AluOpType.is_equal,
                    )

                # phase A: spmm + filter per tile
                stg = smpool.tile([TILE, 4 * ST], F32, tag="stg")
                neis = {}
                if PH < 3:
                    continue
                for t in range(t0, t1):
                    tl = t - t0
                    for dd in range(2):
                        dinfo = c["dirs"][dd]
                        pst = ps_mm.tile([P, TILE], F32, tag="ps_mm")
                        mms = []
                        for h in range(2):
                            nchv = int(plan.nch[dd, t, h])
                            if nchv == 0:
                                continue
                            base = dinfo["mslots"][(t, h)]
                            mms.extend(range(base, base + nchv))
                        for i, m in enumerate(mms):
                            nc.tensor.matmul(
                                out=pst[:],
                                lhsT=m_sb[dd][:, m * CHUNK: (m + 1) * CHUNK],
                                rhs=s_sb[dd][:, m * TILE: (m + 1) * TILE],
                                start=(i == 0),
                                stop=(i == len(mms) - 1),
                            )
                        nb = neipool.tile([P, TILE], BF16, tag="nei")
                        nc.scalar.activation(
                            out=nb[:], in_=pst[:],
                            func=mybir.ActivationFunctionType.Copy,
                        )
                        neis[(tl, dd)] = nb
                    if PH < 4:
                        continue
                    psf = ps_misc.tile([TILE, 4], F32, tag="ps_misc")
                    nc.tensor.matmul(out=psf[:, 0:1], lhsT=neis[(tl, 0)][:],
                                     rhs=wf2[:, 0:1], start=True, stop=True)
                    nc.tensor.matmul(out=psf[:, 1:2], lhsT=neis[(tl, 1)][:],
                                     rhs=wf2[:, 1:2], start=True, stop=True)
                    nc.tensor.matmul(out=psf[:, 2:4],
                                     lhsT=xT[:, t * TILE: (t + 1) * TILE],
                                     rhs=wf2[:], start=True, stop=True)
                    nc.vector.tensor_copy(out=stg[:, 4 * tl: 4 * tl + 4], in_=psf[:])

                # phase B: gates (batched over tiles in stage)
                if PH < 5:
                    continue
                stg_v = stg[:].rearrange("p (t v) -> p t v", v=4)[:, :T_, :]
                gst = smpool.tile([TILE, 5 * ST], F32, tag="gst")
                gst_v = gst[:].rearrange("p (t v) -> p t v", v=5)[:, :T_, :]
                nd = ndat_v[:, t0:t1, :]
                scr = smpool.tile([TILE, 4 * ST], F32, tag="scr")
                scr_v = scr[:].rearrange("p (t v) -> p t v", v=4)[:, :T_, :]
                MUL, ADD, SUB = (mybir.AluOpType.mult, mybir.AluOpType.add,
                                 mybir.AluOpType.subtract)

                def tt(out, a, b, op, eng=None):
                    (eng or nc.vector).tensor_tensor(out=out, in0=a, in1=b, op=op)

                # c_dir = inv*raw - xw + t
                for dd in range(2):
                    tt(scr_v[:, :, dd], stg_v[:, :, dd], nd[:, :, dd], MUL)
                    tt(scr_v[:, :, dd], scr_v[:, :, dd], stg_v[:, :, 2 + dd], SUB)
                    tt(scr_v[:, :, dd], scr_v[:, :, dd], nd[:, :, 2 + dd], ADD)
                    nc.scalar.activation(
                        out=scr_v[:, :, 2 + dd], in_=scr_v[:, :, dd],
                        func=mybir.ActivationFunctionType.Exp,
                    )
                tt(gst_v[:, :, 0], scr_v[:, :, 2], scr_v[:, :, 3], ADD)
                nc.vector.reciprocal(out=gst_v[:, :, 1], in_=gst_v[:, :, 0])
                for dd in range(2):
                    # C = exp/sum ; Cm = C*mask + mask_bias
                    tt(scr_v[:, :, dd], scr_v[:, :, 2 + dd], gst_v[:, :, 1], MUL)
                    tt(scr_v[:, :, dd], scr_v[:, :, dd], nd[:, :, 4 + 2 * dd], MUL)
                    tt(gst_v[:, :, 2 + dd], scr_v[:, :, dd], nd[:, :, 5 + 2 * dd], ADD)
                    # gain = Cm * inv
                    tt(gst_v[:, :, dd], gst_v[:, :, 2 + dd], nd[:, :, dd], MUL)
                nc.vector.memset(gst_v[:, :, 4], 1.0)
                nc.vector.tensor_copy(out=call_o[:, t0:t1], in_=gst_v[:, :, 2])
                nc.vector.tensor_copy(out=call_i[:, t0:t1], in_=gst_v[:, :, 3])

                if PH < 6:
                    continue
                # phase C: per-tile output
                for t in range(t0, t1):
                    tl = t - t0
                    # transpose this tile's 5 gate columns; transpose
                    # outputs must land at PSUM partition 0, so use three
                    # transposes into disjoint column ranges of one tile
                    gt_ps = ps_misc.tile([3, 3 * TILE], F32, tag="ps_misc")
                    nc.tensor.transpose(out=gt_ps[0:1, 0:TILE],
                                        in_=gst[:, 5 * tl: 5 * tl + 1],
                                        identity=ident[:TILE, :TILE])
                    nc.tensor.transpose(out=gt_ps[0:1, TILE: 2 * TILE],
                                        in_=gst[:, 5 * tl + 1: 5 * tl + 2],
                                        identity=ident[:TILE, :TILE])
                    nc.tensor.transpose(out=gt_ps[0:3, 2 * TILE: 3 * TILE],
                                        in_=gst[:, 5 * tl + 2: 5 * tl + 5],
                                        identity=ident[:TILE, :TILE])
                    gt = opool.tile([3, 3 * TILE], BF16, tag="gt")
                    nc.scalar.activation(out=gt[:], in_=gt_ps[:],
                                         func=mybir.ActivationFunctionType.Copy)
                    bc = ps_misc.tile([P, P], F32, tag="ps_misc")
                    for dd in range(2):
                        nc.tensor.matmul(
                            out=bc[:, dd * TILE: (dd + 1) * TILE],
                            lhsT=ones[0:1, :],
                            rhs=gt[0:1, dd * TILE: (dd + 1) * TILE],
                            start=True, stop=True,
                        )
                    ng = []
                    for dd in range(2):
                        g = opool.tile([P, TILE], BF16, tag=f"ng{dd}")
                        tt(g[:], neis[(tl, dd)][:],
                           bc[:, dd * TILE: (dd + 1) * TILE], MUL)
                        ng.append(g)
                    pso = ps_misc.tile([P, TILE], F32, tag="ps_misc")
                    nc.tensor.matmul(out=pso[:], lhsT=wmain[:, 0:D], rhs=ng[0][:],
                                     start=True, stop=False)
                    nc.tensor.matmul(out=pso[:], lhsT=wmain[:, D:2 * D], rhs=ng[1][:],
                                     start=False, stop=False)
                    nc.tensor.matmul(out=pso[:], lhsT=wmain[:, 2 * D:3 * D],
                                     rhs=xT[:, t * TILE: (t + 1) * TILE],
                                     start=False, stop=False)
                    nc.tensor.matmul(out=pso[:], lhsT=b3[:],
                                     rhs=gt[0:3, 2 * TILE: 3 * TILE],
                                     start=False, stop=True)
                    if PH < 7:
                        continue
                    osb = opool.tile([P, TILE], F32, tag="osb")
                    nc.scalar.activation(out=osb[:], in_=pso[:],
                                         func=mybir.ActivationFunctionType.Copy)
                    tr = ps_misc.tile([TILE, P], F32, tag="ps_misc")
                    nc.tensor.transpose(out=tr[:], in_=osb[:], identity=ident[:])
                    onode = opool.tile([TILE, P], F32, tag="onode")
                    nc.vector.tensor_copy(out=onode[:], in_=tr[:])
                    rows = min(TILE, plan.NPC - t * TILE)
                    nc.sync.dma_start(out_d[t * TILE: t * TILE + rows, :],
                                      onode[:rows, :])

            # C_in / C_out outputs
            nfull = plan.NPC // TILE
            for cal, dr in (((call_o, cout_d), (call_i, cin_d)) if PH >= 8 else ()):
                nc.sync.dma_start(
                    dr[: nfull * TILE, 0].rearrange("(t p) -> p t", p=TILE),
                    cal[:, :nfull],
                )
                rem = plan.NPC - nfull * TILE
                if rem:
                    nc.sync.dma_start(
                        dr[nfull * TILE:, 0].rearrange("(t p) -> p t", p=rem),
                        cal[:rem, nfull: nfull + 1],
                    )
    nc.compile()
    return nc


def run(inputs, trace=False, tmpdir=None):
    plan, in_maps = preprocess(**inputs)
    nc = build_kernel(plan)
    res = run_bass_kernel_spmd(nc, in_maps, core_ids=list(range(NCORES)),
                               trace=trace, tmpdir=tmpdir)
    return plan, res


def kernel(**inputs):
    plan, res = run(inputs)
    n, npc = plan.N, plan.NPC
    out = np.empty((n, D), np.float32)
    c_in = np.empty((n, 1), np.float32)
    c_out = np.empty((n, 1), np.float32)
    for k in range(NCORES):
        r = res.results[k]
        out[k * npc: (k + 1) * npc] = r["out"]
        c_out[k * npc: (k + 1) * npc] = r["cout"]
        c_in[k * npc: (k + 1) * npc] = r["cin"]
    return out, c_in, c_out


def assemble(plan, res):
    n, npc = plan.N, plan.NPC
    out = np.empty((n, D), np.float32)
    c_in = np.empty((n, 1), np.float32)
    c_out = np.empty((n, 1), np.float32)
    for k in range(NCORES):
        r = res.results[k]
        out[k * npc: (k + 1) * npc] = r["out"]
        c_out[k * npc: (k + 1) * npc] = r["cout"]
        c_in[k * npc: (k + 1) * npc] = r["cin"]
    return out, c_in, c_out


# revision 12
# speedup vs baseline: 2.8856x; 1.0208x over previous
"""ADiGCNConv distributed Trainium2 kernel (8 NeuronCores).

Strategy
--------
Node-sharded: core k owns destination nodes [k*N/8, (k+1)*N/8).
The directed-GCN normalization is separable:
    out_nei = diag(o_inv) . A  . diag(i_inv) . x
    in_nei  = diag(i_inv) . A^T. diag(o_inv) . x
so the host prescales x into two bf16 gather tables (y_i = i_inv*x,
y_o = o_inv*x, each split into two <=32768-row halves for int16 gather
indices) and the device does, per destination tile of 64 nodes:
  dma_gather of the source rows for each edge (dest-sorted, chunked
  into 128-edge chunks) -> one-hot selection matrices built on DVE via
  broadcast is_equal against an iota -> TensorE matmul segment-sum
  accumulated in PSUM (feature-major) -> fused dense epilogue
  (degree-gated filter, softmax gates, three weight matmuls, bias via
  rank-3 matmul, PE transpose) -> DMA out.
Everything after aggregation is node-local; tables are replicated per
core so no collectives are needed.
"""

import sys

sys.path.insert(0, "/opt/trn_rl_repo")

import numpy as np
import ml_dtypes

import os

import concourse.bacc as bacc
import concourse.mybir as mybir
from concourse.tile import TileContext
from concourse.bass_utils import run_bass_kernel_spmd

P = 128
D = 128
TILE = 64          # destination nodes per PSUM tile
ST = 6             # tiles per stage
CHUNK = 128        # edges per matmul chunk
NCORES = 8

BF16 = mybir.dt.bfloat16
F32 = mybir.dt.float32
I16 = mybir.dt.int16


def _ceil(a, b):
    return (a + b - 1) // b


class Plan:
    """Shared (SPMD-uniform) program structure, data-independent across cores
    (built from the max chunk counts over all cores)."""

    def __init__(self, n, npc):
        self.N = n
        self.NPC = npc
        self.HALF = _ceil(n, 2)
        assert self.HALF <= 32768, "int16 gather index limit"
        self.NT = _ceil(npc, TILE)
        self.NTP = self.NT * TILE
        self.NS = _ceil(self.NT, ST)
        # filled later:
        self.nch = None          # [2, NT, 2] chunks per (dir, tile, half)
        self.gbase = None        # [2, NT, 2] global chunk index of group start
        self.totch = 0
        self.calls = []          # per stage: list of dicts
        self.gcols = 0

    def finalize(self, counts):
        """counts: [ncores, 2, NT, 2] edge counts."""
        mx = counts.max(axis=0)                      # [2, NT, 2]
        nch = _ceil(mx, CHUNK)
        # ensure every (dir, tile) has >= 1 chunk so PSUM gets initialized
        empty = nch.sum(axis=2) == 0                 # [2, NT]
        nch[:, :, 0][empty] = 1
        self.nch = nch
        # global chunk ordering: stage -> dir -> half -> tile -> chunk
        gbase = np.zeros((2, self.NT, 2), np.int64)
        g = 0
        coloff = 0
        self.calls = []
        for s in range(self.NS):
            t0, t1 = s * ST, min((s + 1) * ST, self.NT)
            stage_calls = []
            seg0 = g
            for d in range(2):
                dirinfo = {"halves": [], "ch": 0, "mslots": {}}
                mslot = 0
                for h in range(2):
                    ch = int(nch[d, t0:t1, h].sum())
                    dirinfo["halves"].append(
                        {"ch": ch, "coloff": coloff, "g0": g}
                    )
                    for t in range(t0, t1):
                        gbase[d, t, h] = g
                        dirinfo["mslots"][(t, h)] = mslot
                        g += int(nch[d, t, h])
                        mslot += int(nch[d, t, h])
                    coloff += ch * 8
                dirinfo["ch"] = mslot
                stage_calls.append(dirinfo)
            self.calls.append(
                {"t0": t0, "t1": t1, "dirs": stage_calls, "seg0": seg0, "segn": g - seg0}
            )
        self.gbase = gbase
        self.totch = g
        self.gcols = coloff
        self.max_ch_dir = max(c["dirs"][d]["ch"] for c in self.calls for d in range(2))
        self.max_seg = max(c["segn"] for c in self.calls)


def preprocess(x, edge_index, in_degree, out_degree,
               out_deg_mask, out_deg_mask_bias, in_deg_mask, in_deg_mask_bias,
               W_src, b_src, W_dst, b_dst, W_out_f, b_out_f, W_in_f, b_in_f,
               W_fc, b_fc, in_deg_table, out_deg_table, ncores=NCORES):
    n = x.shape[0]
    npc = n // ncores
    assert npc * ncores == n
    plan = Plan(n, npc)

    row = np.asarray(edge_index[0], np.int64)
    col = np.asarray(edge_index[1], np.int64)
    e = row.shape[0]

    deg_out = np.bincount(row, minlength=n).astype(np.float32)
    deg_in = np.bincount(col, minlength=n).astype(np.float32)
    o_inv = np.where(deg_out > 0, 1.0 / np.sqrt(np.maximum(deg_out, 1e-12)), 0.0).astype(np.float32)
    i_inv = np.where(deg_in > 0, 1.0 / np.sqrt(np.maximum(deg_in, 1e-12)), 0.0).astype(np.float32)

    xf = np.asarray(x, np.float32)
    y_i = (xf * i_inv[:, None]).astype(ml_dtypes.bfloat16)   # gathered for out_nei
    y_o = (xf * o_inv[:, None]).astype(ml_dtypes.bfloat16)   # gathered for in_nei
    H = plan.HALF
    tables = {
        "yi0": np.ascontiguousarray(y_i[:H]),
        "yi1": np.ascontiguousarray(y_i[H:]),
        "yo0": np.ascontiguousarray(y_o[:H]),
        "yo1": np.ascontiguousarray(y_o[H:]),
    }

    # per-core, per-dir edge lists
    percore = []
    counts = np.zeros((ncores, 2, plan.NT, 2), np.int64)
    for k in range(ncores):
        entry = []
        for d in range(2):
            dst = row if d == 0 else col
            src = col if d == 0 else row
            m = (dst >= k * npc) & (dst < (k + 1) * npc)
            dl = dst[m] - k * npc
            sr = src[m]
            tile = dl // TILE
            half = (sr >= H).astype(np.int64)
            seg = dl % TILE
            gid = tile * 2 + half
            cnt = np.bincount(gid, minlength=plan.NT * 2).reshape(plan.NT, 2)
            counts[k, d] = cnt
            entry.append((dl, sr, tile, half, seg, gid))
        percore.append(entry)
    plan.finalize(counts)

    # degree-gated filter per-node terms (host lookup of tiny tables)
    w_out_vec = (out_deg_table.astype(np.float64) @ W_out_f[0].astype(np.float64))
    w_in_vec = (in_deg_table.astype(np.float64) @ W_in_f[0].astype(np.float64))
    t_out_all = (w_out_vec[np.asarray(out_degree)] + float(b_out_f[0])).astype(np.float32)
    t_in_all = (w_in_vec[np.asarray(in_degree)] + float(b_in_f[0])).astype(np.float32)

    # weights (device constants, shared across cores)
    wf2 = np.stack([W_out_f[0], W_in_f[0]], axis=1).astype(ml_dtypes.bfloat16)  # [128,2]
    wmain = np.concatenate(
        [W_src.T, W_dst.T, 0.5 * W_fc.T], axis=1
    ).astype(ml_dtypes.bfloat16)                                                # [128,384]
    b3rows = np.stack([b_src, b_dst, 0.5 * b_fc], axis=0).astype(np.float32)    # [3,128]
    b3 = b3rows.astype(ml_dtypes.bfloat16)
    iota = np.tile(np.arange(TILE, dtype=np.float32), (P, 1)).astype(ml_dtypes.bfloat16)
    ident = np.eye(P, dtype=np.float32)
    ones = np.ones((1, P), dtype=ml_dtypes.bfloat16)
    consts = {"wf2": wf2, "wmain": wmain, "b3": b3, "iota": iota,
              "ident": ident, "ones": ones}

    # per-core input arrays
    in_maps = []
    for k in range(ncores):
        seg_flat = np.full(plan.totch * CHUNK, -1.0, np.float32)
        idx_flat = np.zeros(plan.totch * CHUNK, np.int64)
        for d in range(2):
            dl, sr, tile, half, seg, gid = percore[k][d]
            order = np.argsort(gid, kind="stable")
            gids = gid[order]
            # rank within group
            cnt = counts[k, d].reshape(-1)
            starts = np.zeros(plan.NT * 2, np.int64)
            starts[1:] = np.cumsum(cnt)[:-1]
            rank = np.arange(gids.shape[0]) - starts[gids]
            gb = plan.gbase[d].reshape(-1)  # [NT*2]
            pos = gb[gids] * CHUNK + rank
            seg_flat[pos] = seg[order]
            idx_flat[pos] = sr[order] - half[order] * H
        gseg = np.ascontiguousarray(
            seg_flat.reshape(plan.totch, CHUNK).T
        ).astype(ml_dtypes.bfloat16)
        # gather index layout per call: [16, ch*8] wrapped, replicated x8
        gidx = np.zeros((P, plan.gcols), np.int16)
        for s in range(plan.NS):
            for d in range(2):
                for h in range(2):
                    hinfo = plan.calls[s]["dirs"][d]["halves"][h]
                    ch, co, g0 = hinfo["ch"], hinfo["coloff"], hinfo["g0"]
                    if ch == 0:
                        continue
                    vals = idx_flat[g0 * CHUNK: (g0 + ch) * CHUNK]
                    arr = vals.reshape(ch * 8, 16).T.astype(np.int16)
                    for g in range(8):
                        gidx[g * 16:(g + 1) * 16, co: co + ch * 8] = arr

        nsl = slice(k * npc, (k + 1) * npc)
        nodedat = np.zeros((plan.NTP, 8), np.float32)
        nodedat[:npc, 0] = o_inv[nsl]
        nodedat[:npc, 1] = i_inv[nsl]
        nodedat[:npc, 2] = t_out_all[nsl]
        nodedat[:npc, 3] = t_in_all[nsl]
        nodedat[:npc, 4] = np.asarray(out_deg_mask, np.float32)[nsl]
        nodedat[:npc, 5] = np.asarray(out_deg_mask_bias, np.float32)[nsl]
        nodedat[:npc, 6] = np.asarray(in_deg_mask, np.float32)[nsl]
        nodedat[:npc, 7] = np.asarray(in_deg_mask_bias, np.float32)[nsl]

        xT = np.zeros((P, plan.NTP), np.float32)
        xT[:, :npc] = xf[nsl].T
        xT = xT.astype(ml_dtypes.bfloat16)

        im = {"gseg": gseg, "gidx": gidx, "nodedat": nodedat, "xT": xT}
        im.update(tables)
        im.update(consts)
        in_maps.append(im)
    return plan, in_maps


PH = int(os.environ.get("GNN_PHASE", "9"))


class StopBuild(Exception):
    pass


def build_kernel(plan):
    nc = bacc.Bacc("TRN2", target_bir_lowering=False, debug=False,
                   num_swdge_queues=4)
    H, H2 = plan.HALF, plan.N - plan.HALF
    tabs = {}
    for nm, rows in (("yi0", H), ("yi1", H2), ("yo0", H), ("yo1", H2)):
        tabs[nm] = nc.dram_tensor(nm, [rows, D], BF16, kind="ExternalInput")
    gseg_d = nc.dram_tensor("gseg", [P, plan.totch], BF16, kind="ExternalInput")
    gidx_d = nc.dram_tensor("gidx", [P, plan.gcols], I16, kind="ExternalInput")
    ndat_d = nc.dram_tensor("nodedat", [plan.NTP, 8], F32, kind="ExternalInput")
    xT_d = nc.dram_tensor("xT", [P, plan.NTP], BF16, kind="ExternalInput")
    wf2_d = nc.dram_tensor("wf2", [D, 2], BF16, kind="ExternalInput")
    wmain_d = nc.dram_tensor("wmain", [D, 3 * D], BF16, kind="ExternalInput")
    b3_d = nc.dram_tensor("b3", [3, D], BF16, kind="ExternalInput")
    iota_d = nc.dram_tensor("iota", [P, TILE], BF16, kind="ExternalInput")
    ident_d = nc.dram_tensor("ident", [P, P], F32, kind="ExternalInput")
    ones_d = nc.dram_tensor("ones", [1, P], BF16, kind="ExternalInput")
    out_d = nc.dram_tensor("out", [plan.NPC, D], F32, kind="ExternalOutput")
    cout_d = nc.dram_tensor("cout", [plan.NPC, 1], F32, kind="ExternalOutput")
    cin_d = nc.dram_tensor("cin", [plan.NPC, 1], F32, kind="ExternalOutput")

    max_cols = max(
        sum(h["ch"] for d in c["dirs"] for h in d["halves"]) * 8 for c in plan.calls
    )

    with TileContext(nc) as tc:
        with (
            tc.tile_pool(name="const", bufs=1) as cpool,
            tc.tile_pool(name="stream", bufs=3) as spool,
            tc.tile_pool(name="nei", bufs=4 * ST) as neipool,
            tc.tile_pool(name="small", bufs=2) as smpool,
            tc.tile_pool(name="ops", bufs=3) as opool,
            tc.tile_pool(name="ps_mm", bufs=4, space="PSUM") as ps_mm,
            tc.tile_pool(name="ps_misc", bufs=4, space="PSUM") as ps_misc,
        ):
            # resident constants
            wf2 = cpool.tile([D, 2], BF16, tag="wf2")
            wmain = cpool.tile([D, 3 * D], BF16, tag="wmain")
            b3 = cpool.tile([3, D], BF16, tag="b3")
            iota = cpool.tile([P, TILE], BF16, tag="iota")
            ident = cpool.tile([P, P], F32, tag="ident")
            ones = cpool.tile([1, P], BF16, tag="ones")
            xT = cpool.tile([P, plan.NTP], BF16, tag="xT")
            ndat = cpool.tile([TILE, plan.NT * 8], F32, tag="ndat")
            call_o = cpool.tile([TILE, plan.NT], F32, tag="call_o")
            call_i = cpool.tile([TILE, plan.NT], F32, tag="call_i")
            for sb, dr in ((wf2, wf2_d), (wmain, wmain_d), (b3, b3_d),
                           (iota, iota_d), (ident, ident_d), (ones, ones_d),
                           (xT, xT_d)):
                nc.sync.dma_start(sb[:], dr[:])
            nc.sync.dma_start(
                ndat[:].rearrange("p (t v) -> p t v", v=8),
                ndat_d[:].rearrange("(t p) v -> p t v", p=TILE),
            )

            ndat_v = ndat[:].rearrange("p (t v) -> p t v", v=8)

            for s in range(plan.NS):
                c = plan.calls[s]
                t0, t1 = c["t0"], c["t1"]
                T_ = t1 - t0
                # stage streaming inputs
                col0 = c["dirs"][0]["halves"][0]["coloff"]
                ncols = sum(h["ch"] for dd in c["dirs"] for h in dd["halves"]) * 8
                idx_sb = spool.tile([P, max_cols], I16, tag="idx")
                nc.sync.dma_start(idx_sb[:, :ncols], gidx_d[:, col0: col0 + ncols])
                seg_sb = spool.tile([P, plan.max_seg], BF16, tag="seg")
                nc.sync.dma_start(
                    seg_sb[:, : c["segn"]],
                    gseg_d[:, c["seg0"]: c["seg0"] + c["segn"]],
                )

                m_sb = []
                s_sb = []
                qn = 0
                for dd in range(2):
                    dinfo = c["dirs"][dd]
                    ch_dir = dinfo["ch"]
                    mt = spool.tile([P, plan.max_ch_dir * CHUNK], BF16, tag=f"m{dd}")
                    m_sb.append(mt)
                    for h in range(2):
                        hi = dinfo["halves"][h]
                        if hi["ch"] == 0:
                            continue
                        tab = tabs[("yi0", "yi1", "yo0", "yo1")[dd * 2 + h]]
                        moff = (hi["g0"] - dinfo["halves"][0]["g0"]) if h else 0
                        nidx = hi["ch"] * CHUNK
                        if PH < 1:
                            continue
                        nc.gpsimd.dma_gather(
                            mt[:, moff * CHUNK: (moff + hi["ch"]) * CHUNK]
                            .rearrange("p (c e) -> p c e", e=D),
                            tab[:],
                            idx_sb[:, hi["coloff"] - col0: hi["coloff"] - col0 + hi["ch"] * 8],
                            nidx,
                            nidx,
                            D,
                            single_packet=False,
                            queue_num=qn % 4,
                        )
                        qn += 1
                    # S build for the whole (stage, dir)
                    st = spool.tile([P, plan.max_ch_dir * TILE], BF16, tag=f"s{dd}")
                    s_sb.append(st)
                    soff = dinfo["halves"][0]["g0"] - c["seg0"]
                    if PH < 2:
                        nc.vector.memset(st[:, : ch_dir * TILE], 0.0)
                        continue
                    nc.vector.tensor_tensor(
                        out=st[:, : ch_dir * TILE].rearrange("p (c d) -> p c d", d=TILE),
                        in0=seg_sb[:, soff: soff + ch_dir]
                        .unsqueeze(2).to_broadcast([P, ch_dir, TILE]),
                        in1=iota[:].unsqueeze(1).to_broadcast([P, ch_dir, TILE]),
                        op=mybir.AluOpType.is_equal,
                    )

                # phase A: spmm + filter per tile
                stg = smpool.tile([TILE, 4 * ST], F32, tag="stg")
                neis = {}
                if PH < 3:
                    continue
                for t in range(t0, t1):
                    tl = t - t0
                    for dd in range(2):
                        dinfo = c["dirs"][dd]
                        pst = ps_mm.tile([P, TILE], F32, tag="ps_mm")
                        mms = []
                        for h in range(2):
                            nchv = int(plan.nch[dd, t, h])
                            if nchv == 0:
                                continue
                            base = dinfo["mslots"][(t, h)]
                            mms.extend(range(base, base + nchv))
                        for i, m in enumerate(mms):
                            nc.tensor.matmul(
                                out=pst[:],
                                lhsT=m_sb[dd][:, m * CHUNK: (m + 1) * CHUNK],
                                rhs=s_sb[dd][:, m * TILE: (m + 1) * TILE],
                                start=(i == 0),
                                stop=(i == len(mms) - 1),
                            )
                        nb = neipool.tile([P, TILE], BF16, tag="nei")
                        nc.scalar.activation(
                            out=nb[:], in_=pst[:],
                            func=mybir.ActivationFunctionType.Copy,
                        )
                        neis[(tl, dd)] = nb
                    if PH < 4:
                        continue
                    psf = ps_misc.tile([TILE, 4], F32, tag="ps_misc")
                    nc.tensor.matmul(out=psf[:, 0:1], lhsT=neis[(tl, 0)][:],
                                     rhs=wf2[:, 0:1], start=True, stop=True)
                    nc.tensor.matmul(out=psf[:, 1:2], lhsT=neis[(tl, 1)][:],
                                     rhs=wf2[:, 1:2], start=True, stop=True)
                    nc.tensor.matmul(out=psf[:, 2:4],
                                     lhsT=xT[:, t * TILE: (t + 1) * TILE],
                                     rhs=wf2[:], start=True, stop=True)
                    nc.vector.tensor_copy(out=stg[:, 4 * tl: 4 * tl + 4], in_=psf[:])

                # phase B: gates (batched over tiles in stage)
                if PH < 5:
                    continue
                stg_v = stg[:].rearrange("p (t v) -> p t v", v=4)[:, :T_, :]
                gst = smpool.tile([TILE, 5 * ST], F32, tag="gst")
                gst_v = gst[:].rearrange("p (t v) -> p t v", v=5)[:, :T_, :]
                nd = ndat_v[:, t0:t1, :]
                scr = smpool.tile([TILE, 4 * ST], F32, tag="scr")
                scr_v = scr[:].rearrange("p (t v) -> p t v", v=4)[:, :T_, :]
                MUL, ADD, SUB = (mybir.AluOpType.mult, mybir.AluOpType.add,
                                 mybir.AluOpType.subtract)

                def tt(out, a, b, op, eng=None):
                    (eng or nc.vector).tensor_tensor(out=out, in0=a, in1=b, op=op)

                # c_dir = inv*raw - xw + t
                for dd in range(2):
                    tt(scr_v[:, :, dd], stg_v[:, :, dd], nd[:, :, dd], MUL)
                    tt(scr_v[:, :, dd], scr_v[:, :, dd], stg_v[:, :, 2 + dd], SUB)
                    tt(scr_v[:, :, dd], scr_v[:, :, dd], nd[:, :, 2 + dd], ADD)
                    nc.scalar.activation(
                        out=scr_v[:, :, 2 + dd], in_=scr_v[:, :, dd],
                        func=mybir.ActivationFunctionType.Exp,
                    )
                tt(gst_v[:, :, 0], scr_v[:, :, 2], scr_v[:, :, 3], ADD)
                nc.vector.reciprocal(out=gst_v[:, :, 1], in_=gst_v[:, :, 0])
                for dd in range(2):
                    # C = exp/sum ; Cm = C*mask + mask_bias
                    tt(scr_v[:, :, dd], scr_v[:, :, 2 + dd], gst_v[:, :, 1], MUL)
                    tt(scr_v[:, :, dd], scr_v[:, :, dd], nd[:, :, 4 + 2 * dd], MUL)
                    tt(gst_v[:, :, 2 + dd], scr_v[:, :, dd], nd[:, :, 5 + 2 * dd], ADD)
                    # gain = Cm * inv
                    tt(gst_v[:, :, dd], gst_v[:, :, 2 + dd], nd[:, :, dd], MUL)
                nc.vector.memset(gst_v[:, :, 4], 1.0)
                nc.vector.tensor_copy(out=call_o[:, t0:t1], in_=gst_v[:, :, 2])
                nc.vector.tensor_copy(out=call_i[:, t0:t1], in_=gst_v[:, :, 3])

                if PH < 6:
                    continue
                # phase C: per-tile output
                for t in range(t0, t1):
                    tl = t - t0
                    # transpose this tile's 5 gate columns; transpose
                    # outputs must land at PSUM partition 0, so use three
                    # transposes into disjoint column ranges of one tile
                    gt_ps = ps_misc.tile([3, 3 * TILE], F32, tag="ps_misc")
                    nc.tensor.transpose(out=gt_ps[0:1, 0:TILE],
                                        in_=gst[:, 5 * tl: 5 * tl + 1],
                                        identity=ident[:TILE, :TILE])
                    nc.tensor.transpose(out=gt_ps[0:1, TILE: 2 * TILE],
                                        in_=gst[:, 5 * tl + 1: 5 * tl + 2],
                                        identity=ident[:TILE, :TILE])
                    nc.tensor.transpose(out=gt_ps[0:3, 2 * TILE: 3 * TILE],
                                        in_=gst[:, 5 * tl + 2: 5 * tl + 5],
                                        identity=ident[:TILE, :TILE])
                    gt = opool.tile([3, 3 * TILE], BF16, tag="gt")
                    nc.scalar.activation(out=gt[:], in_=gt_ps[:],
                                         func=mybir.ActivationFunctionType.Copy)
                    bc = ps_misc.tile([P, P], F32, tag="ps_misc")
                    for dd in range(2):
                        nc.tensor.matmul(
                            out=bc[:, dd * TILE: (dd + 1) * TILE],
                            lhsT=ones[0:1, :],
                            rhs=gt[0:1, dd * TILE: (dd + 1) * TILE],
                            start=True, stop=True,
                        )
                    ng = []
                    for dd in range(2):
                        g = opool.tile([P, TILE], BF16, tag=f"ng{dd}")
                        tt(g[:], neis[(tl, dd)][:],
                           bc[:, dd * TILE: (dd + 1) * TILE], MUL)
                        ng.append(g)
                    pso = ps_misc.tile([P, TILE], F32, tag="ps_misc")
                    nc.tensor.matmul(out=pso[:], lhsT=wmain[:, 0:D], rhs=ng[0][:],
                                     start=True, stop=False)
                    nc.tensor.matmul(out=pso[:], lhsT=wmain[:, D:2 * D], rhs=ng[1][:],
                                     start=False, stop=False)
                    nc.tensor.matmul(out=pso[:], lhsT=wmain[:, 2 * D:3 * D],
                                     rhs=xT[:, t * TILE: (t + 1) * TILE],
                                     start=False, stop=False)
                    nc.tensor.matmul(out=pso[:], lhsT=b3[:],
                                     rhs=gt[0:3, 2 * TILE: 3 * TILE],
                                     start=False, stop=True)
                    if PH < 7:
                        continue
                    osb = opool.tile([P, TILE], F32, tag="osb")
                    nc.scalar.activation(out=osb[:], in_=pso[:],
                                         func=mybir.ActivationFunctionType.Copy)
                    tr = ps_misc.tile([TILE, P], F32, tag="ps_misc")
                    nc.tensor.transpose(out=tr[:], in_=osb[:], identity=ident[:])
                    onode = opool.tile([TILE, P], F32, tag="onode")
                    nc.vector.tensor_copy(out=onode[:], in_=tr[:])
                    rows = min(TILE, plan.NPC - t * TILE)
                    nc.sync.dma_start(out_d[t * TILE: t * TILE + rows, :],
                                      onode[:rows, :])

            # C_in / C_out outputs
            nfull = plan.NPC // TILE
            for cal, dr in (((call_o, cout_d), (call_i, cin_d)) if PH >= 8 else ()):
                nc.sync.dma_start(
                    dr[: nfull * TILE, 0].rearrange("(t p) -> p t", p=TILE),
                    cal[:, :nfull],
                )
                rem = plan.NPC - nfull * TILE
                if rem:
                    nc.sync.dma_start(
                        dr[nfull * TILE:, 0].rearrange("(t p) -> p t", p=rem),
                        cal[:rem, nfull: nfull + 1],
                    )
    nc.compile()
    return nc


def run(inputs, trace=False, tmpdir=None):
    plan, in_maps = preprocess(**inputs)
    nc = build_kernel(plan)
    res = run_bass_kernel_spmd(nc, in_maps, core_ids=list(range(NCORES)),
                               trace=trace, tmpdir=tmpdir)
    return plan, res


def kernel(**inputs):
    plan, res = run(inputs)
    n, npc = plan.N, plan.NPC
    out = np.empty((n, D), np.float32)
    c_in = np.empty((n, 1), np.float32)
    c_out = np.empty((n, 1), np.float32)
    for k in range(NCORES):
        r = res.results[k]
        out[k * npc: (k + 1) * npc] = r["out"]
        c_out[k * npc: (k + 1) * npc] = r["cout"]
        c_in[k * npc: (k + 1) * npc] = r["cin"]
    return out, c_in, c_out


def assemble(plan, res):
    n, npc = plan.N, plan.NPC
    out = np.empty((n, D), np.float32)
    c_in = np.empty((n, 1), np.float32)
    c_out = np.empty((n, 1), np.float32)
    for k in range(NCORES):
        r = res.results[k]
        out[k * npc: (k + 1) * npc] = r["out"]
        c_out[k * npc: (k + 1) * npc] = r["cout"]
        c_in[k * npc: (k + 1) * npc] = r["cin"]
    return out, c_in, c_out
